# revision 1
# baseline (speedup 1.0000x reference)
"""GNN message passing (gather + weighted scatter-add) on 8 Trainium2 cores.

out[n, f] = sum over edges e with dst[e]==n of edge_weight[e] * x[src[e], f]

Strategy:
  - Destination-shard: core c owns output nodes [c*12500, (c+1)*12500). No
    collectives needed; host concatenates the 8 output slices.
  - Host packs each core's edges sorted by (dst_tile, src), padding each
    tile block to a multiple of 128 with zero-weight dummy edges, and to
    identical block sizes across cores so all 8 cores run one SPMD program.
  - Device: indirect DMA (DynamicDMA) gathers x rows (256B each) from HBM
    into SBUF in matmul-ready [128, k, 64] layout: row for chunk-slot (p, j)
    = x[idx[p, j]]. For every 128-edge chunk, VectorE builds a weighted
    one-hot selection matrix ((iota == dst_local) * w) and TensorE
    accumulates onehot.T @ x_rows into a PSUM tile per 128-node output tile.
    ScalarE evacuates PSUM into an SBUF output buffer, DMA streams it out.
"""

import math
import numpy as np

N = 100000
E = 1000000
F = 64
NCORES = 8
NPC = N // NCORES            # nodes per core
TILE = 128
NT = math.ceil(NPC / TILE)   # output tiles per core (98)
B = 14                       # tiles per pass
NPASS = math.ceil(NT / B)    # 7

MM_DT = "float16"            # matmul dtype: "float32" or "float16" or "bfloat16"

DBG_NO_GATHER = False        # replace gather with memset (bisection)
REPEAT = 1                   # repeat device compute (timing amplification)


def pack_host(x, edge_weight, edge_index):
    """Returns (shared schedule, per-core tables)."""
    src = np.asarray(edge_index[0], dtype=np.int64)
    dst = np.asarray(edge_index[1], dtype=np.int64)
    w = np.asarray(edge_weight, dtype=np.float32)

    core = dst // NPC
    counts = np.zeros((NCORES, NT), dtype=np.int64)
    percore = []
    for c in range(NCORES):
        sel = core == c
        es = src[sel]
        ed = dst[sel] - c * NPC
        ew = w[sel]
        t = ed >> 7
        order = np.lexsort((es, t))
        es, ed, ew, t = es[order], ed[order], ew[order], t[order]
        np.add.at(counts[c], t, 1)
        percore.append((es, ed, ew, t))

    K = (np.ceil(counts.max(axis=0) / TILE)).astype(np.int64)  # [NT] chunks/tile
    L = K * TILE
    off = np.zeros(NT, dtype=np.int64)
    off[1:] = np.cumsum(L)[:-1]
    Ltot = int(L.sum())
    NC = Ltot // TILE  # total matmul chunks

    sched_t = np.repeat(np.arange(NT), K)  # tile of each chunk

    # per-pass chunk-column ranges
    pass_cols = np.zeros((NPASS, 2), dtype=np.int64)
    run = 0
    for p in range(NPASS):
        t0, t1 = p * B, min((p + 1) * B, NT)
        n = int(K[t0:t1].sum())
        pass_cols[p] = (run, run + n)
        run += n

    tables = []
    for c in range(NCORES):
        es, ed, ew, t = percore[c]
        # rank of each edge within its tile block
        changes = np.empty(len(t), dtype=bool)
        changes[0] = True
        if len(t) > 1:
            changes[1:] = t[1:] != t[:-1]
        starts = np.flatnonzero(changes)
        rank = np.arange(len(t)) - np.repeat(starts, np.diff(np.append(starts, len(t))))
        pos = off[t] + rank

        src32 = np.zeros(Ltot, dtype=np.int32)
        dstf = np.zeros(Ltot, dtype=np.float32)
        wf = np.zeros(Ltot, dtype=np.float32)
        src32[pos] = es.astype(np.int32)
        dstf[pos] = (ed - t * TILE).astype(np.float32)
        wf[pos] = ew

        # [128, NC] tables: column cc serves matmul chunk cc, partition = edge slot
        idx_tbl = np.ascontiguousarray(src32.reshape(NC, TILE).T)
        dst_tbl = np.ascontiguousarray(dstf.reshape(NC, TILE).T)
        w_tbl = np.ascontiguousarray(wf.reshape(NC, TILE).T)
        tables.append((idx_tbl, dst_tbl, w_tbl))

    sched = dict(K=K, NC=NC, pass_cols=pass_cols, sched_t=sched_t)
    return sched, tables


def emulate_core(sched, table, x):
    """Numpy emulation of the device program for one core (packing check)."""
    idx_tbl, dst_tbl, w_tbl = table
    NCc = sched["NC"]
    iota = np.arange(TILE, dtype=np.float32)
    out = np.zeros((NT * TILE, F), dtype=np.float32)
    for cc in range(NCc):
        t = sched["sched_t"][cc]
        xg = x[idx_tbl[:, cc]]                                      # [128, 64]
        oh = (iota[None, :] == dst_tbl[:, cc, None]) * w_tbl[:, cc, None]
        out[t * TILE:(t + 1) * TILE] += oh.T @ xg
    return out[:NPC]


WAIT_CAPS = {
    "InstEventSemaphore": 8,
}


def split_excess_waits(nc):
    """Walrus only encodes one sync wait per instruction (for most ISA
    structs). Move the excess onto standalone InstEventSemaphore
    instructions placed just before, in the same engine stream —
    same-engine waiting earlier is always safe. Also fills the ISA bytes
    of library-reload pseudo-instructions (raw-Bass path leaves them
    empty and walrus rejects that)."""
    import concourse.mybir as mybir
    n = 0
    for f in nc.m.functions:
        for bb in f.blocks:
            for ins in bb.instructions:
                if type(ins).__name__ == "InstPseudoReloadLibraryIndex" and not ins.instr:
                    b = [0] * 64
                    b[0], b[1], b[12], b[16] = 223, 16, 2, int(ins.lib_index)
                    ins.instr = b
            # dedicated scratch sem per engine for inert ES updates --
            # ids 245..250 are beyond anything Tile allocates
            eng_ids = {}
            new = []
            for ins in bb.instructions:
                si = ins.sync_info
                waits = list(si.on_wait) if (si is not None and si.on_wait) else []
                cap = WAIT_CAPS.get(type(ins).__name__, 1)
                if len(waits) > cap:
                    excess, keep = waits[:-cap], waits[-cap:]
                    if ins.engine not in eng_ids:
                        eng_ids[ins.engine] = 245 + len(eng_ids)
                    sem_id = eng_ids[ins.engine]
                    sem_name = f"esw_scratch_{sem_id}"
                    for wchunk in [excess[i:i + 1] for i in range(len(excess))]:
                        n += 1
                        # inert 0-add update on the engine's own sem: race
                        # detector / cost model require every instruction to
                        # update something, and same-engine updates can't race
                        upd = mybir.SyncUpdate(
                            sync_type="semaphore", id=sem_id, ant_name=sem_name,
                            update_mode="sem-add-imm", update_value=0,
                        )
                        es = mybir.InstEventSemaphore(
                            name=f"ESW-{n}-{ins.name}",
                            engine=ins.engine,
                            ins=[], outs=[],
                            sync_info=mybir.SyncInfo(on_wait=wchunk, on_update=[upd]),
                        )
                        new.append(es)
                    si.on_wait = keep
                new.append(ins)
            bb.instructions = new
    return n


_walrus_patched = False


def patch_walrus_dge():
    """Add --dge-levels so walrus lowers vector-dynamic-offset (indirect)
    DMAs; without it DynamicDMA is disabled and the gather silently no-ops."""
    global _walrus_patched
    if _walrus_patched:
        return
    import concourse.bass_utils as bu
    orig = bu.run_command

    def run_command_dge(argv, **kw):
        argv = list(argv)
        if argv and "walrus_driver" in str(argv[0]) and not any(
                str(a).startswith("--dge-levels") for a in argv):
            argv.append("--dge-levels=vector_dynamic_offsets")
        return orig(argv, **kw)

    bu.run_command = run_command_dge
    _walrus_patched = True


def build_bass(sched, mm_dt_name=MM_DT):
    import concourse.bass as bass
    import concourse.mybir as mybir
    import concourse.tile as tile

    patch_walrus_dge()

    f32 = mybir.dt.float32
    mm_dt = getattr(mybir.dt, mm_dt_name)
    K = sched["K"]; NC = sched["NC"]
    pass_cols = sched["pass_cols"]

    nc = bass.Bass("TRN2")
    x_d = nc.dram_tensor("x", [N, F], f32, kind="ExternalInput")
    idx_d = nc.dram_tensor("idx", [128, NC], mybir.dt.int32, kind="ExternalInput")
    # merged f32 const table: [dstf | wf | iota] so one DMA covers all consts
    ftbl_d = nc.dram_tensor("ftbl", [128, 2 * NC + 128], f32, kind="ExternalInput")
    out_d = nc.dram_tensor("out", [NT * TILE, F], f32, kind="ExternalOutput")

    colsmax = int(max(pass_cols[p, 1] - pass_cols[p, 0] for p in range(NPASS)))

    with tile.TileContext(nc, pool_alloc_mode="queue") as tc:
        with (
            tc.tile_pool(name="const", bufs=1) as constp,
            tc.tile_pool(name="xg", bufs=8) as xgp,
            tc.tile_pool(name="cast", bufs=8) as castp,
            tc.tile_pool(name="oh", bufs=8) as ohp,
            tc.tile_pool(name="outb", bufs=2) as outp,
            tc.tile_pool(name="psum", bufs=4, space="PSUM") as psump,
        ):
            ftbl_sb = constp.tile([128, 2 * NC + 128], f32, tag="ftbl")
            nc.sync.dma_start(ftbl_sb[:], ftbl_d[:])
            iota_sb = constp.tile([128, 128], mm_dt, tag="iota")
            nc.vector.tensor_copy(iota_sb[:], ftbl_sb[:, 2 * NC:2 * NC + 128])
            idx_sb = constp.tile([128, NC], mybir.dt.int32, tag="idx")
            nc.sync.dma_start(idx_sb[:], idx_d[:])

            for _rep in range(REPEAT):
              cc = 0
              for p in range(NPASS):
                t0, t1 = p * B, min((p + 1) * B, NT)
                ob = outp.tile([128, (t1 - t0) * F], f32, tag="outb")
                for t in range(t0, t1):
                    ktot = int(K[t])
                    if ktot == 0:
                        nc.vector.memset(ob[:, (t - t0) * F:(t - t0 + 1) * F], 0.0)
                        continue
                    ps = psump.tile([128, F], f32, tag="ps")
                    for k in range(ktot):
                        xt = xgp.tile([128, F], f32, tag="xg")
                        if DBG_NO_GATHER:
                            nc.gpsimd.memset(xt[:], 1.0)
                        else:
                            nc.gpsimd.indirect_dma_start(
                                out=xt[:], out_offset=None, in_=x_d[:],
                                in_offset=bass.IndirectOffsetOnAxis(
                                    ap=idx_sb[:, cc:cc + 1], axis=0),
                            )
                        if mm_dt_name == "float32":
                            rhs = xt
                        else:
                            rhs = castp.tile([128, F], mm_dt, tag="cast")
                            nc.scalar.copy(rhs[:], xt[:])
                        oh = ohp.tile([128, 128], mm_dt, tag="oh")
                        # weighted one-hot: (iota == dst_local) * w, fused
                        nc.vector.tensor_scalar(
                            oh[:], iota_sb[:],
                            ftbl_sb[:, cc:cc + 1], ftbl_sb[:, NC + cc:NC + cc + 1],
                            op0=mybir.AluOpType.is_equal, op1=mybir.AluOpType.mult,
                        )
                        nc.tensor.matmul(
                            ps[:], lhsT=oh[:], rhs=rhs[:],
                            start=(k == 0), stop=(k == ktot - 1),
                        )
                        cc += 1
                    nc.scalar.copy(ob[:, (t - t0) * F:(t - t0 + 1) * F], ps[:])
                dview = out_d[t0 * TILE:t1 * TILE, :].rearrange("(t q) f -> q t f", q=128)
                nc.sync.dma_start(dview, ob[:].rearrange("q (t f) -> q t f", f=F))
            assert cc == NC
    nsplit = split_excess_waits(nc)
    print(f"split_excess_waits: {nsplit} waits moved to event-semaphore instrs")
    return nc


def make_in_maps(sched, tables, x):
    iota_np = np.arange(128, dtype=np.float32)[None, :].repeat(128, axis=0)
    in_maps = []
    for c in range(NCORES):
        idx_tbl, dst_tbl, w_tbl = tables[c]
        ftbl = np.ascontiguousarray(
            np.concatenate([dst_tbl, w_tbl, iota_np], axis=1), dtype=np.float32)
        in_maps.append({"x": x, "idx": idx_tbl, "ftbl": ftbl})
    return in_maps


def kernel(x, edge_weight, edge_index, num_nodes):
    x = np.ascontiguousarray(np.asarray(x, dtype=np.float32))
    sched, tables = pack_host(x, edge_weight, edge_index)
    nc = build_bass(sched)
    in_maps = make_in_maps(sched, tables, x)

    from concourse.bass_utils import run_bass_kernel_spmd
    res = run_bass_kernel_spmd(nc, in_maps, core_ids=list(range(NCORES)))
    out = np.concatenate([res.results[c]["out"][:NPC] for c in range(NCORES)], axis=0)
    return out.astype(np.float32)



# revision 3
# speedup vs baseline: 5.5044x; 5.5044x over previous
"""GNN message passing (gather + weighted scatter-add) on 8 Trainium2 cores.

out[n, f] = sum over edges e with dst[e]==n of edge_weight[e] * x[src[e], f]

Strategy:
  - Destination-shard: core c owns output nodes [c*12500, (c+1)*12500). No
    collectives needed; host concatenates the 8 output slices.
  - Host packs each core's edges sorted by (dst_tile, src), padding each
    tile block to a multiple of 128 with zero-weight dummy edges, and to
    identical block sizes across cores so all 8 cores run one SPMD program.
  - Device: indirect DMA (DynamicDMA) gathers x rows (256B each) from HBM
    into SBUF in matmul-ready [128, k, 64] layout: row for chunk-slot (p, j)
    = x[idx[p, j]]. For every 128-edge chunk, VectorE builds a weighted
    one-hot selection matrix ((iota == dst_local) * w) and TensorE
    accumulates onehot.T @ x_rows into a PSUM tile per 128-node output tile.
    ScalarE evacuates PSUM into an SBUF output buffer, DMA streams it out.
"""

import math
import numpy as np

N = 100000
E = 1000000
F = 64
NCORES = 8
NPC = N // NCORES            # nodes per core
TILE = 128
NT = math.ceil(NPC / TILE)   # output tiles per core (98)
B = 14                       # tiles per pass
NPASS = math.ceil(NT / B)    # 7

MM_DT = "float16"            # matmul dtype: "float32" or "float16" or "bfloat16"

DBG_NO_GATHER = False        # replace gather with memset (bisection)
REPEAT = 1                   # repeat device compute (timing amplification)


def pack_host(x, edge_weight, edge_index):
    """Returns (shared schedule, per-core tables)."""
    src = np.asarray(edge_index[0], dtype=np.int64)
    dst = np.asarray(edge_index[1], dtype=np.int64)
    w = np.asarray(edge_weight, dtype=np.float32)

    core = dst // NPC
    counts = np.zeros((NCORES, NT), dtype=np.int64)
    percore = []
    for c in range(NCORES):
        sel = core == c
        es = src[sel]
        ed = dst[sel] - c * NPC
        ew = w[sel]
        t = ed >> 7
        order = np.lexsort((es, t))
        es, ed, ew, t = es[order], ed[order], ew[order], t[order]
        np.add.at(counts[c], t, 1)
        percore.append((es, ed, ew, t))

    K = (np.ceil(counts.max(axis=0) / TILE)).astype(np.int64)  # [NT] chunks/tile
    L = K * TILE
    off = np.zeros(NT, dtype=np.int64)
    off[1:] = np.cumsum(L)[:-1]
    Ltot = int(L.sum())
    NC = Ltot // TILE  # total matmul chunks

    sched_t = np.repeat(np.arange(NT), K)  # tile of each chunk

    # per-pass chunk-column ranges
    pass_cols = np.zeros((NPASS, 2), dtype=np.int64)
    run = 0
    for p in range(NPASS):
        t0, t1 = p * B, min((p + 1) * B, NT)
        n = int(K[t0:t1].sum())
        pass_cols[p] = (run, run + n)
        run += n

    tables = []
    for c in range(NCORES):
        es, ed, ew, t = percore[c]
        # rank of each edge within its tile block
        changes = np.empty(len(t), dtype=bool)
        changes[0] = True
        if len(t) > 1:
            changes[1:] = t[1:] != t[:-1]
        starts = np.flatnonzero(changes)
        rank = np.arange(len(t)) - np.repeat(starts, np.diff(np.append(starts, len(t))))
        pos = off[t] + rank

        src32 = np.zeros(Ltot, dtype=np.int32)
        dstf = np.zeros(Ltot, dtype=np.float32)
        wf = np.zeros(Ltot, dtype=np.float32)
        src32[pos] = es.astype(np.int32)
        dstf[pos] = (ed - t * TILE).astype(np.float32)
        wf[pos] = ew

        # [128, NC] tables: column cc serves matmul chunk cc, partition = edge slot
        idx_tbl = np.ascontiguousarray(src32.reshape(NC, TILE).T)
        dst_tbl = np.ascontiguousarray(dstf.reshape(NC, TILE).T)
        w_tbl = np.ascontiguousarray(wf.reshape(NC, TILE).T)
        tables.append((idx_tbl, dst_tbl, w_tbl))

    sched = dict(K=K, NC=NC, pass_cols=pass_cols, sched_t=sched_t)
    return sched, tables


def emulate_core(sched, table, x):
    """Numpy emulation of the device program for one core (packing check)."""
    idx_tbl, dst_tbl, w_tbl = table
    NCc = sched["NC"]
    iota = np.arange(TILE, dtype=np.float32)
    out = np.zeros((NT * TILE, F), dtype=np.float32)
    for cc in range(NCc):
        t = sched["sched_t"][cc]
        xg = x[idx_tbl[:, cc]]                                      # [128, 64]
        oh = (iota[None, :] == dst_tbl[:, cc, None]) * w_tbl[:, cc, None]
        out[t * TILE:(t + 1) * TILE] += oh.T @ xg
    return out[:NPC]


WAIT_CAPS = {
    "InstEventSemaphore": 8,
}


def split_excess_waits(nc):
    """Walrus only encodes one sync wait per instruction (for most ISA
    structs). Move the excess onto standalone InstEventSemaphore
    instructions placed just before, in the same engine stream —
    same-engine waiting earlier is always safe. Also fills the ISA bytes
    of library-reload pseudo-instructions (raw-Bass path leaves them
    empty and walrus rejects that)."""
    import concourse.mybir as mybir
    n = 0
    for f in nc.m.functions:
        for bb in f.blocks:
            for ins in bb.instructions:
                if type(ins).__name__ == "InstPseudoReloadLibraryIndex" and not ins.instr:
                    b = [0] * 64
                    b[0], b[1], b[12], b[16] = 223, 16, 2, int(ins.lib_index)
                    ins.instr = b
            # dedicated scratch sem per engine for inert ES updates --
            # ids 245..250 are beyond anything Tile allocates
            eng_ids = {}
            new = []
            for ins in bb.instructions:
                si = ins.sync_info
                waits = list(si.on_wait) if (si is not None and si.on_wait) else []
                cap = WAIT_CAPS.get(type(ins).__name__, 1)
                if len(waits) > cap:
                    excess, keep = waits[:-cap], waits[-cap:]
                    if ins.engine not in eng_ids:
                        eng_ids[ins.engine] = 245 + len(eng_ids)
                    sem_id = eng_ids[ins.engine]
                    sem_name = f"esw_scratch_{sem_id}"
                    for wchunk in [excess[i:i + 1] for i in range(len(excess))]:
                        n += 1
                        # inert 0-add update on the engine's own sem: race
                        # detector / cost model require every instruction to
                        # update something, and same-engine updates can't race
                        upd = mybir.SyncUpdate(
                            sync_type="semaphore", id=sem_id, ant_name=sem_name,
                            update_mode="sem-add-imm", update_value=0,
                        )
                        es = mybir.InstEventSemaphore(
                            name=f"ESW-{n}-{ins.name}",
                            engine=ins.engine,
                            ins=[], outs=[],
                            sync_info=mybir.SyncInfo(on_wait=wchunk, on_update=[upd]),
                        )
                        new.append(es)
                    si.on_wait = keep
                new.append(ins)
            bb.instructions = new
    return n


_walrus_patched = False


def patch_walrus_dge():
    """Add --dge-levels so walrus lowers vector-dynamic-offset (indirect)
    DMAs; without it DynamicDMA is disabled and the gather silently no-ops."""
    global _walrus_patched
    if _walrus_patched:
        return
    import concourse.bass_utils as bu
    orig = bu.run_command

    def run_command_dge(argv, **kw):
        argv = list(argv)
        if argv and "walrus_driver" in str(argv[0]) and not any(
                str(a).startswith("--dge-levels") for a in argv):
            argv.append("--dge-levels=vector_dynamic_offsets")
        return orig(argv, **kw)

    bu.run_command = run_command_dge
    _walrus_patched = True


def build_bass(sched, mm_dt_name=MM_DT):
    import concourse.bass as bass
    import concourse.mybir as mybir
    import concourse.tile as tile

    patch_walrus_dge()

    f32 = mybir.dt.float32
    mm_dt = getattr(mybir.dt, mm_dt_name)
    K = sched["K"]; NC = sched["NC"]
    pass_cols = sched["pass_cols"]

    nc = bass.Bass("TRN2")
    x_d = nc.dram_tensor("x", [N, F], f32, kind="ExternalInput")
    idx_d = nc.dram_tensor("idx", [128, NC], mybir.dt.int32, kind="ExternalInput")
    # merged f32 const table: [dstf | wf | iota] so one DMA covers all consts
    ftbl_d = nc.dram_tensor("ftbl", [128, 2 * NC + 128], f32, kind="ExternalInput")
    out_d = nc.dram_tensor("out", [NT * TILE, F], f32, kind="ExternalOutput")

    colsmax = int(max(pass_cols[p, 1] - pass_cols[p, 0] for p in range(NPASS)))

    with tile.TileContext(nc, pool_alloc_mode="queue") as tc:
        with (
            tc.tile_pool(name="const", bufs=1) as constp,
            tc.tile_pool(name="xg", bufs=8) as xgp,
            tc.tile_pool(name="cast", bufs=8) as castp,
            tc.tile_pool(name="oh", bufs=8) as ohp,
            tc.tile_pool(name="outb", bufs=2) as outp,
            tc.tile_pool(name="psum", bufs=4, space="PSUM") as psump,
        ):
            ftbl_sb = constp.tile([128, 2 * NC + 128], f32, tag="ftbl")
            nc.sync.dma_start(ftbl_sb[:], ftbl_d[:])
            iota_sb = constp.tile([128, 128], mm_dt, tag="iota")
            nc.vector.tensor_copy(iota_sb[:], ftbl_sb[:, 2 * NC:2 * NC + 128])
            idx_sb = constp.tile([128, NC], mybir.dt.int32, tag="idx")
            nc.sync.dma_start(idx_sb[:], idx_d[:])

            for _rep in range(REPEAT):
              cc = 0
              for p in range(NPASS):
                t0, t1 = p * B, min((p + 1) * B, NT)
                ob = outp.tile([128, (t1 - t0) * F], f32, tag="outb")
                for t in range(t0, t1):
                    ktot = int(K[t])
                    if ktot == 0:
                        nc.vector.memset(ob[:, (t - t0) * F:(t - t0 + 1) * F], 0.0)
                        continue
                    ps = psump.tile([128, F], f32, tag="ps")
                    for k in range(ktot):
                        xt = xgp.tile([128, F], f32, tag="xg")
                        if DBG_NO_GATHER:
                            nc.gpsimd.memset(xt[:], 1.0)
                        else:
                            nc.gpsimd.indirect_dma_start(
                                out=xt[:], out_offset=None, in_=x_d[:],
                                in_offset=bass.IndirectOffsetOnAxis(
                                    ap=idx_sb[:, cc:cc + 1], axis=0),
                            )
                        if mm_dt_name == "float32":
                            rhs = xt
                        else:
                            rhs = castp.tile([128, F], mm_dt, tag="cast")
                            nc.scalar.copy(rhs[:], xt[:])
                        oh = ohp.tile([128, 128], mm_dt, tag="oh")
                        # weighted one-hot: (iota == dst_local) * w, fused
                        nc.vector.tensor_scalar(
                            oh[:], iota_sb[:],
                            ftbl_sb[:, cc:cc + 1], ftbl_sb[:, NC + cc:NC + cc + 1],
                            op0=mybir.AluOpType.is_equal, op1=mybir.AluOpType.mult,
                        )
                        nc.tensor.matmul(
                            ps[:], lhsT=oh[:], rhs=rhs[:],
                            start=(k == 0), stop=(k == ktot - 1),
                        )
                        cc += 1
                    nc.scalar.copy(ob[:, (t - t0) * F:(t - t0 + 1) * F], ps[:])
                dview = out_d[t0 * TILE:t1 * TILE, :].rearrange("(t q) f -> q t f", q=128)
                nc.sync.dma_start(dview, ob[:].rearrange("q (t f) -> q t f", f=F))
            assert cc == NC
    nsplit = split_excess_waits(nc)
    print(f"split_excess_waits: {nsplit} waits moved to event-semaphore instrs")
    return nc


def make_in_maps(sched, tables, x):
    iota_np = np.arange(128, dtype=np.float32)[None, :].repeat(128, axis=0)
    in_maps = []
    for c in range(NCORES):
        idx_tbl, dst_tbl, w_tbl = tables[c]
        ftbl = np.ascontiguousarray(
            np.concatenate([dst_tbl, w_tbl, iota_np], axis=1), dtype=np.float32)
        in_maps.append({"x": x, "idx": idx_tbl, "ftbl": ftbl})
    return in_maps


def kernel(x, edge_weight, edge_index, num_nodes):
    x = np.ascontiguousarray(np.asarray(x, dtype=np.float32))
    sched, tables = pack_host(x, edge_weight, edge_index)
    nc = build_bass(sched)
    in_maps = make_in_maps(sched, tables, x)

    from concourse.bass_utils import run_bass_kernel_spmd
    res = run_bass_kernel_spmd(nc, in_maps, core_ids=list(range(NCORES)))
    out = np.concatenate([res.results[c]["out"][:NPC] for c in range(NCORES)], axis=0)
    return out.astype(np.float32)



# revision 6
# speedup vs baseline: 5.9876x; 1.0878x over previous
"""GNN message passing (gather + weighted scatter-add) on 8 Trainium2 cores, v2.

out[n, f] = sum over edges e with dst[e]==n of edge_weight[e] * x[src[e], f]

Architecture (driven by measured per-instruction dispatch costs of ~30-120us
on this runtime — total instruction count is everything):
  - Destination-shard: core c owns output nodes [c*12500, (c+1)*12500).
  - x lives fully SBUF-resident, fp16, transposed + quarter-partitioned:
    xT4[p, n, l] = x[(p//32)*25000 + n, 2*(p%32) + l]  -> [128, 25000, 2],
    100KB/partition. int16 ap_gather indices stay < 25000.
  - Edges packed into 8-slot rows per dst node (ceil(deg/8) rows/node).
    Slot j: ap_gather pulls x columns for src_j into every partition
    (each 16-partition group uses its own index: the src quarter that
    group holds, else a dummy); host-baked weights wt4[p, j] =
    w_j * (quarter(p) == quarter(src_j)) kill the wrong-quarter copies.
  - Per chunk (~6-7k slots): 1 ap_gather (gpsimd) + 1 broadcast multiply
    (DVE) + 1 strided row-reduce (DVE) + wt/rowsum DMAs. ~30 chunks/core,
    ~130 instructions total.
  - Row sums [128=(pair,quarter), rows, 2] stream to DRAM; the host sums
    the <=4 quarter partials and ceil(deg/8) row partials per node (O(N)
    work) and re-interleaves features.
"""

import math
import numpy as np

N = 100000
E = 1000000
F = 64
NCORES = 8
NPC = N // NCORES            # 12500 dst nodes per core
Q = 4                        # x quarters (int16 index limit)
NQ = N // Q                  # 25000
NQ1 = NQ + 1                 # +1 zero-sentinel column (gathered by dummies)
L = 4                        # slots per row
CHUNK = 7168                 # slots per chunk (multiple of 128)

MULT_MODE = "dvebc"          # "dvebc" (bcast+DVE) | "gpsimd" (bcast+Pool ALU)
REPEAT = 1                   # device-body repetitions (timing amplification)


def pack_host(x, edge_weight, edge_index):
    src = np.asarray(edge_index[0], dtype=np.int64)
    dst = np.asarray(edge_index[1], dtype=np.int64)
    w = np.asarray(edge_weight, dtype=np.float32)

    xpair = np.ascontiguousarray(
        x.astype(np.float16).reshape(N, 32, 2).transpose(1, 0, 2)
    )  # [32, N, 2]
    xt4 = np.zeros((128, NQ1, 2), dtype=np.float16)
    xt4[:, 1:, :] = (
        xpair.reshape(32, Q, NQ, 2).transpose(1, 0, 2, 3).reshape(128, NQ, 2)
    )  # partition p = (q = p//32, pair = p%32); column 0 stays zero (sentinel)

    core = dst // NPC
    cores = []
    for c in range(NCORES):
        sel = core == c
        es = src[sel]
        ed = dst[sel] - c * NPC
        ew = w[sel]
        order = np.argsort(ed, kind="stable")
        es, ed, ew = es[order], ed[order], ew[order]

        deg = np.bincount(ed, minlength=NPC)
        nrows_per_node = np.maximum((deg + L - 1) // L, 1)
        nrows = int(nrows_per_node.sum())
        # chunk layout in rows (CHUNK/L rows per chunk), pad rows to fill
        rows_per_chunk = CHUNK // L
        nchunks = math.ceil(nrows / rows_per_chunk)
        nrows_pad = nchunks * rows_per_chunk
        nslots = nrows_pad * L

        row_node = np.zeros(nrows_pad, dtype=np.int64)  # node of each row
        # rows in node order
        row_node[:nrows] = np.repeat(np.arange(NPC), nrows_per_node)
        row_node[nrows:] = 0  # pad rows -> node 0 with zero weight

        slot_src = np.zeros(nslots, dtype=np.int64)
        slot_w = np.zeros(nslots, dtype=np.float32)
        # edge positions: node n's edges go into its rows' slots in order
        row_start = np.zeros(NPC + 1, dtype=np.int64)
        row_start[1:] = np.cumsum(nrows_per_node)
        node_edge_start = np.zeros(NPC + 1, dtype=np.int64)
        node_edge_start[1:] = np.cumsum(deg)
        # position of edge within its node  (edges are dst-sorted)
        epos = np.arange(len(ed)) - node_edge_start[ed]
        slot_idx = row_start[ed] * L + epos
        slot_src[slot_idx] = es
        slot_w[slot_idx] = ew

        sq = slot_src // NQ          # quarter of each slot's src
        sl = (slot_src % NQ + 1).astype(np.int16)  # 1-based; 0 = zero sentinel

        # ap_gather index table [128, nslots/16] int16, per-16-partition group:
        # the group holding quarter q gathers its slots' rows; others gather 0s
        idx16 = np.zeros((128, nslots // 16), dtype=np.int16)
        slocal = sl.reshape(nslots // 16, 16).T  # [16, s]
        squar = sq.reshape(nslots // 16, 16).T
        for g in range(8):
            gq = g // 2
            idx16[g * 16:(g + 1) * 16, :] = np.where(squar == gq, slocal, 0)

        # quarter-masked per-slot weights [128, nslots] fp16:
        # wt4[p, j] = w_j if p//32 == quarter(src_j) else 0
        wrow = slot_w.astype(np.float16)
        wt4 = np.zeros((128, nslots), dtype=np.float16)
        for q in range(Q):
            wt4[q * 32:(q + 1) * 32, :] = np.where(sq == q, wrow, np.float16(0))

        cores.append(dict(
            idx16=idx16, wt4=wt4, nrows=nrows, nrows_pad=nrows_pad,
            nslots=nslots, nchunks=nchunks, row_node=row_node,
        ))
    maxchunks = max(c["nchunks"] for c in cores)
    # pad all cores to identical chunk count (single SPMD program)
    for c in cores:
        if c["nchunks"] < maxchunks:
            extra = (maxchunks - c["nchunks"]) * CHUNK
            c["idx16"] = np.concatenate(
                [c["idx16"], np.zeros((128, extra // 16), np.int16)], axis=1)
            c["wt4"] = np.concatenate(
                [c["wt4"], np.zeros((128, extra), np.float16)], axis=1)
            pad_rows = extra // L
            c["row_node"] = np.concatenate(
                [c["row_node"], np.zeros(pad_rows, np.int64)])
            c["nchunks"] = maxchunks
            c["nrows_pad"] += pad_rows
            c["nslots"] += extra
    return xt4, cores, maxchunks


def emulate_core(xt4, core):
    """Numpy emulation of the device program for one core."""
    nch = core["nchunks"]
    idx16, wt4 = core["idx16"], core["wt4"]
    rows_out = np.zeros((128, core["nrows_pad"], 2), dtype=np.float16)
    for ch in range(nch):
        s0 = ch * CHUNK
        g = np.zeros((128, CHUNK, 2), dtype=np.float16)
        for grp in range(8):
            idxs = idx16[grp * 16:(grp + 1) * 16, s0 // 16:(s0 + CHUNK) // 16]
            flat = idxs.T.reshape(-1).astype(np.int64)  # slot order
            g[grp * 16:(grp + 1) * 16] = xt4[grp * 16:(grp + 1) * 16, flat, :]
        wc = wt4[:, s0:s0 + CHUNK, None].astype(np.float32)
        gw = (g.astype(np.float32) * wc).astype(np.float16)
        r = gw.astype(np.float32).reshape(128, CHUNK // L, L, 2).sum(axis=2)
        rows_out[:, s0 // L:(s0 + CHUNK) // L, :] = r.astype(np.float16)
    return rows_out


def combine_host(rows_out, core):
    """rows [128=(q,pair), rows, 2] fp16 -> [12500, 64] f32 for one core."""
    r = rows_out.astype(np.float32)  # [128, R, 2]
    rq = r.reshape(Q, 32, -1, 2).sum(axis=0)  # [32, R, 2]
    out = np.zeros((NPC, 32, 2), dtype=np.float32)
    np.add.at(out, core["row_node"], rq.transpose(1, 0, 2))
    return out.reshape(NPC, F)


WAIT_CAPS = {"InstEventSemaphore": 8}


def split_excess_waits(nc):
    """Walrus only encodes one sync wait per instruction; move the excess
    onto standalone InstEventSemaphore instructions placed just before."""
    import concourse.mybir as mybir
    n = 0
    for f in nc.m.functions:
        for bb in f.blocks:
            eng_ids = {}
            new = []
            for ins in bb.instructions:
                si = ins.sync_info
                waits = list(si.on_wait) if (si is not None and si.on_wait) else []
                cap = WAIT_CAPS.get(type(ins).__name__, 1)
                if len(waits) > cap:
                    excess, keep = waits[:-cap], waits[-cap:]
                    if ins.engine not in eng_ids:
                        eng_ids[ins.engine] = 245 + len(eng_ids)
                    sem_id = eng_ids[ins.engine]
                    sem_name = f"esw_scratch_{sem_id}"
                    for wchunk in [excess[i:i + 1] for i in range(len(excess))]:
                        n += 1
                        upd = mybir.SyncUpdate(
                            sync_type="semaphore", id=sem_id, ant_name=sem_name,
                            update_mode="sem-add-imm", update_value=0,
                        )
                        es = mybir.InstEventSemaphore(
                            name=f"ESW-{n}-{ins.name}",
                            engine=ins.engine,
                            ins=[], outs=[],
                            sync_info=mybir.SyncInfo(on_wait=wchunk, on_update=[upd]),
                        )
                        new.append(es)
                    si.on_wait = keep
                new.append(ins)
            bb.instructions = new
    return n


_walrus_patched = False


def patch_walrus_dge():
    global _walrus_patched
    if _walrus_patched:
        return
    import concourse.bass_utils as bu
    orig = bu.run_command

    def run_command_dge(argv, **kw):
        argv = list(argv)
        if argv and "walrus_driver" in str(argv[0]) and not any(
                str(a).startswith("--dge-levels") for a in argv):
            argv.append("--dge-levels=vector_dynamic_offsets")
        return orig(argv, **kw)

    bu.run_command = run_command_dge
    _walrus_patched = True


def build_bass(nchunks, nslots):
    import concourse.bass as bass
    import concourse.mybir as mybir
    import concourse.tile as tile
    from concourse import library_config
    from concourse.library_overlay import lower_extended_insts

    patch_walrus_dge()
    f16, i16 = mybir.dt.float16, mybir.dt.int16
    nrows_pad = nslots // L

    nc = bass.Bass("TRN2")
    xt4_d = nc.dram_tensor("xt4", [128, NQ1, 2], f16, kind="ExternalInput")
    idx_d = nc.dram_tensor("idx16", [128, nslots // 16], i16, kind="ExternalInput")
    wt4_d = nc.dram_tensor("wt4", [128, nslots], f16, kind="ExternalInput")
    rows_d = nc.dram_tensor("rows", [128, nrows_pad, 2], f16, kind="ExternalOutput")

    with nc.allow_low_precision("fp16 8-term row sums; host combines in f32"):
      with tile.TileContext(nc, pool_alloc_mode="queue") as tc:
        with (
            tc.tile_pool(name="const", bufs=1) as constp,
            tc.tile_pool(name="g", bufs=2) as gp,
            tc.tile_pool(name="wt", bufs=2) as wtp,
            tc.tile_pool(name="ix", bufs=2) as ixp,
            tc.tile_pool(name="rs", bufs=2) as rsp,
        ):
            xt4_sb = constp.tile([128, NQ1, 2], f16, tag="xt4")
            nc.sync.dma_start(xt4_sb[:], xt4_d[:])
            nc.gpsimd.load_library(library_config.ap_gather)

            for _rep in range(REPEAT):
                for ch in range(nchunks):
                    s0 = ch * CHUNK
                    idxc = ixp.tile([128, CHUNK // 16], i16, tag="ix")
                    nc.scalar.dma_start(
                        idxc[:], idx_d[:, s0 // 16:(s0 + CHUNK) // 16])
                    g = gp.tile([128, CHUNK, 2], f16, tag="g")
                    nc.gpsimd.ap_gather(
                        g[:], xt4_sb[:], idxc[:],
                        channels=128, num_elems=NQ1, d=2, num_idxs=CHUNK)
                    wtc = wtp.tile([128, CHUNK], f16, tag="wt")
                    nc.scalar.dma_start(wtc[:], wt4_d[:, s0:s0 + CHUNK])
                    mult_eng = (nc.vector if MULT_MODE == "dvebc"
                                else nc.gpsimd)
                    mult_eng.tensor_tensor(
                        out=g[:], in0=g[:],
                        in1=wtc[:].unsqueeze(2).broadcast_to([128, CHUNK, 2]),
                        op=mybir.AluOpType.mult)
                    rs = rsp.tile([128, CHUNK // L, 2], f16, tag="rs")
                    nc.vector.tensor_reduce(
                        out=rs[:],
                        in_=g[:].rearrange("p (r k) two -> p r two k", k=L),
                        axis=mybir.AxisListType.X, op=mybir.AluOpType.add)
                    nc.sync.dma_start(
                        rows_d[:, s0 // L:(s0 + CHUNK) // L, :], rs[:])

    lower_extended_insts(nc)
    split_excess_waits(nc)
    return nc


def kernel(x, edge_weight, edge_index, num_nodes):
    x = np.ascontiguousarray(np.asarray(x, dtype=np.float32))
    xt4, cores, nchunks = pack_host(x, edge_weight, edge_index)
    nslots = cores[0]["nslots"]
    nc = build_bass(nchunks, nslots)
    in_maps = [
        {"xt4": xt4, "idx16": c["idx16"], "wt4": c["wt4"]} for c in cores
    ]
    from concourse.bass_utils import run_bass_kernel_spmd
    res = run_bass_kernel_spmd(nc, in_maps, core_ids=list(range(NCORES)))
    outs = [combine_host(res.results[c]["rows"], cores[c])
            for c in range(NCORES)]
    return np.concatenate(outs, axis=0).astype(np.float32)


# revision 8
# speedup vs baseline: 7.5426x; 1.2597x over previous
"""GNN message passing (gather + weighted scatter-add) on 8 Trainium2 cores, v2.

out[n, f] = sum over edges e with dst[e]==n of edge_weight[e] * x[src[e], f]

Architecture (driven by measured per-instruction dispatch costs of ~30-120us
on this runtime — total instruction count is everything):
  - Destination-shard: core c owns output nodes [c*12500, (c+1)*12500).
  - x lives fully SBUF-resident, fp16, transposed + quarter-partitioned:
    xT4[p, n, l] = x[(p//32)*25000 + n, 2*(p%32) + l]  -> [128, 25000, 2],
    100KB/partition. int16 ap_gather indices stay < 25000.
  - Edges packed into 8-slot rows per dst node (ceil(deg/8) rows/node).
    Slot j: ap_gather pulls x columns for src_j into every partition
    (each 16-partition group uses its own index: the src quarter that
    group holds, else a dummy); host-baked weights wt4[p, j] =
    w_j * (quarter(p) == quarter(src_j)) kill the wrong-quarter copies.
  - Per chunk (~6-7k slots): 1 ap_gather (gpsimd) + 1 broadcast multiply
    (DVE) + 1 strided row-reduce (DVE) + wt/rowsum DMAs. ~30 chunks/core,
    ~130 instructions total.
  - Row sums [128=(pair,quarter), rows, 2] stream to DRAM; the host sums
    the <=4 quarter partials and ceil(deg/8) row partials per node (O(N)
    work) and re-interleaves features.
"""

import math
import numpy as np

N = 100000
E = 1000000
F = 64
NCORES = 8
NPC = N // NCORES            # 12500 dst nodes per core
Q = 4                        # x quarters (int16 index limit)
NQ = N // Q                  # 25000
NQ1 = NQ + 1                 # +1 zero-sentinel column (gathered by dummies)
L = 4                        # slots per row
CHUNK = 6656                 # slots per chunk (multiple of 128)

MULT_MODE = "dve"            # "dve" | "gpsimd" | "split" (alternate)
REPEAT = 1                   # device-body repetitions (timing amplification)


def pack_host(x, edge_weight, edge_index):
    src = np.asarray(edge_index[0], dtype=np.int64)
    dst = np.asarray(edge_index[1], dtype=np.int64)
    w = np.asarray(edge_weight, dtype=np.float32)

    xpair = np.ascontiguousarray(
        x.astype(np.float16).reshape(N, 32, 2).transpose(1, 0, 2)
    )  # [32, N, 2]
    xt4 = np.zeros((128, NQ1, 2), dtype=np.float16)
    xt4[:, 1:, :] = (
        xpair.reshape(32, Q, NQ, 2).transpose(1, 0, 2, 3).reshape(128, NQ, 2)
    )  # partition p = (q = p//32, pair = p%32); column 0 stays zero (sentinel)

    core = dst // NPC
    cores = []
    for c in range(NCORES):
        sel = core == c
        es = src[sel]
        ed = dst[sel] - c * NPC
        ew = w[sel]
        order = np.argsort(ed, kind="stable")
        es, ed, ew = es[order], ed[order], ew[order]

        deg = np.bincount(ed, minlength=NPC)
        nrows_per_node = np.maximum((deg + L - 1) // L, 1)
        nrows = int(nrows_per_node.sum())
        # chunk layout in rows (CHUNK/L rows per chunk), pad rows to fill
        rows_per_chunk = CHUNK // L
        nchunks = math.ceil(nrows / rows_per_chunk)
        nrows_pad = nchunks * rows_per_chunk
        nslots = nrows_pad * L

        row_node = np.zeros(nrows_pad, dtype=np.int64)  # node of each row
        # rows in node order
        row_node[:nrows] = np.repeat(np.arange(NPC), nrows_per_node)
        row_node[nrows:] = 0  # pad rows -> node 0 with zero weight

        slot_src = np.zeros(nslots, dtype=np.int64)
        slot_w = np.zeros(nslots, dtype=np.float32)
        # edge positions: node n's edges go into its rows' slots in order
        row_start = np.zeros(NPC + 1, dtype=np.int64)
        row_start[1:] = np.cumsum(nrows_per_node)
        node_edge_start = np.zeros(NPC + 1, dtype=np.int64)
        node_edge_start[1:] = np.cumsum(deg)
        # position of edge within its node  (edges are dst-sorted)
        epos = np.arange(len(ed)) - node_edge_start[ed]
        slot_idx = row_start[ed] * L + epos
        slot_src[slot_idx] = es
        slot_w[slot_idx] = ew

        sq = slot_src // NQ          # quarter of each slot's src
        sl = (slot_src % NQ + 1).astype(np.int16)  # 1-based; 0 = zero sentinel

        # ap_gather index table [128, nslots/16] int16, per-16-partition group:
        # the group holding quarter q gathers its slots' rows; others gather 0s
        idx16 = np.zeros((128, nslots // 16), dtype=np.int16)
        slocal = sl.reshape(nslots // 16, 16).T  # [16, s]
        squar = sq.reshape(nslots // 16, 16).T
        for g in range(8):
            gq = g // 2
            idx16[g * 16:(g + 1) * 16, :] = np.where(squar == gq, slocal, 0)

        # per-slot weights [1, nslots] fp16 (quarter masking is redundant:
        # wrong-quarter partitions gather the zero sentinel column)
        w1 = slot_w.astype(np.float16)[None, :]

        cores.append(dict(
            idx16=idx16, w1=w1, nrows=nrows, nrows_pad=nrows_pad,
            nslots=nslots, nchunks=nchunks, row_node=row_node,
        ))
    maxchunks = max(c["nchunks"] for c in cores)
    # pad all cores to identical chunk count (single SPMD program)
    for c in cores:
        if c["nchunks"] < maxchunks:
            extra = (maxchunks - c["nchunks"]) * CHUNK
            c["idx16"] = np.concatenate(
                [c["idx16"], np.zeros((128, extra // 16), np.int16)], axis=1)
            c["w1"] = np.concatenate(
                [c["w1"], np.zeros((1, extra), np.float16)], axis=1)
            pad_rows = extra // L
            c["row_node"] = np.concatenate(
                [c["row_node"], np.zeros(pad_rows, np.int64)])
            c["nchunks"] = maxchunks
            c["nrows_pad"] += pad_rows
            c["nslots"] += extra
    return xt4, cores, maxchunks


def emulate_core(xt4, core):
    """Numpy emulation of the device program for one core."""
    nch = core["nchunks"]
    idx16, w1 = core["idx16"], core["w1"]
    rows_out = np.zeros((128, core["nrows_pad"], 2), dtype=np.float16)
    for ch in range(nch):
        s0 = ch * CHUNK
        g = np.zeros((128, CHUNK, 2), dtype=np.float16)
        for grp in range(8):
            idxs = idx16[grp * 16:(grp + 1) * 16, s0 // 16:(s0 + CHUNK) // 16]
            flat = idxs.T.reshape(-1).astype(np.int64)  # slot order
            g[grp * 16:(grp + 1) * 16] = xt4[grp * 16:(grp + 1) * 16, flat, :]
        wc = w1[0, s0:s0 + CHUNK, None].astype(np.float32)
        gw = (g.astype(np.float32) * wc[None]).astype(np.float16)
        r = gw.astype(np.float32).reshape(128, CHUNK // L, L, 2).sum(axis=2)
        rows_out[:, s0 // L:(s0 + CHUNK) // L, :] = r.astype(np.float16)
    return rows_out


def combine_host(rows_out, core):
    """rows [128=(q,pair), rows, 2] fp16 -> [12500, 64] f32 for one core."""
    r = rows_out.astype(np.float32)  # [128, R, 2]
    rq = r.reshape(Q, 32, -1, 2).sum(axis=0)  # [32, R, 2]
    out = np.zeros((NPC, 32, 2), dtype=np.float32)
    np.add.at(out, core["row_node"], rq.transpose(1, 0, 2))
    return out.reshape(NPC, F)


WAIT_CAPS = {"InstEventSemaphore": 8}


def split_excess_waits(nc):
    """Walrus only encodes one sync wait per instruction; move the excess
    onto standalone InstEventSemaphore instructions placed just before."""
    import concourse.mybir as mybir
    n = 0
    for f in nc.m.functions:
        for bb in f.blocks:
            eng_ids = {}
            new = []
            for ins in bb.instructions:
                si = ins.sync_info
                waits = list(si.on_wait) if (si is not None and si.on_wait) else []
                cap = WAIT_CAPS.get(type(ins).__name__, 1)
                if len(waits) > cap:
                    excess, keep = waits[:-cap], waits[-cap:]
                    if ins.engine not in eng_ids:
                        eng_ids[ins.engine] = 245 + len(eng_ids)
                    sem_id = eng_ids[ins.engine]
                    sem_name = f"esw_scratch_{sem_id}"
                    for wchunk in [excess[i:i + 1] for i in range(len(excess))]:
                        n += 1
                        upd = mybir.SyncUpdate(
                            sync_type="semaphore", id=sem_id, ant_name=sem_name,
                            update_mode="sem-add-imm", update_value=0,
                        )
                        es = mybir.InstEventSemaphore(
                            name=f"ESW-{n}-{ins.name}",
                            engine=ins.engine,
                            ins=[], outs=[],
                            sync_info=mybir.SyncInfo(on_wait=wchunk, on_update=[upd]),
                        )
                        new.append(es)
                    si.on_wait = keep
                new.append(ins)
            bb.instructions = new
    return n


_walrus_patched = False


def patch_walrus_dge():
    global _walrus_patched
    if _walrus_patched:
        return
    import concourse.bass_utils as bu
    orig = bu.run_command

    def run_command_dge(argv, **kw):
        argv = list(argv)
        if argv and "walrus_driver" in str(argv[0]) and not any(
                str(a).startswith("--dge-levels") for a in argv):
            argv.append("--dge-levels=vector_dynamic_offsets")
        return orig(argv, **kw)

    bu.run_command = run_command_dge
    _walrus_patched = True


def build_bass(nchunks, nslots):
    import concourse.bass as bass
    import concourse.mybir as mybir
    import concourse.tile as tile
    from concourse import library_config
    from concourse.library_overlay import lower_extended_insts

    patch_walrus_dge()
    f16, i16 = mybir.dt.float16, mybir.dt.int16
    nrows_pad = nslots // L

    nc = bass.Bass("TRN2")
    xt4_d = nc.dram_tensor("xt4", [128, NQ1, 2], f16, kind="ExternalInput")
    idx_d = nc.dram_tensor("idx16", [128, nslots // 16], i16, kind="ExternalInput")
    w1_d = nc.dram_tensor("w1", [1, nslots], f16, kind="ExternalInput")
    rows_d = nc.dram_tensor("rows", [128, nrows_pad, 2], f16, kind="ExternalOutput")

    with nc.allow_low_precision("fp16 8-term row sums; host combines in f32"):
      with tile.TileContext(nc, pool_alloc_mode="queue") as tc:
        with (
            tc.tile_pool(name="const", bufs=1) as constp,
            tc.tile_pool(name="g", bufs=2) as gp,
            tc.tile_pool(name="wt", bufs=2) as wtp,
            tc.tile_pool(name="rs", bufs=2) as rsp,
        ):
            xt4_sb = constp.tile([128, NQ1, 2], f16, tag="xt4")
            nc.sync.dma_start(xt4_sb[:], xt4_d[:])
            idx_sb = constp.tile([128, nslots // 16], i16, tag="idx")
            nc.sync.dma_start(idx_sb[:], idx_d[:])
            nc.gpsimd.load_library(library_config.ap_gather)

            for _rep in range(REPEAT):
                for ch in range(nchunks):
                    s0 = ch * CHUNK
                    g = gp.tile([128, CHUNK, 2], f16, tag="g")
                    nc.gpsimd.ap_gather(
                        g[:], xt4_sb[:], idx_sb[:, s0 // 16:(s0 + CHUNK) // 16],
                        channels=128, num_elems=NQ1, d=2, num_idxs=CHUNK)
                    wtc = wtp.tile([128, CHUNK], f16, tag="wt")
                    nc.scalar.dma_start(
                        wtc[:], w1_d[0:1, s0:s0 + CHUNK].broadcast_to([128, CHUNK]))
                    if MULT_MODE == "dve":
                        mult_eng = nc.vector
                    elif MULT_MODE == "gpsimd":
                        mult_eng = nc.gpsimd
                    else:
                        mult_eng = nc.vector if ch % 2 == 0 else nc.gpsimd
                    mult_eng.tensor_tensor(
                        out=g[:], in0=g[:],
                        in1=wtc[:].unsqueeze(2).broadcast_to([128, CHUNK, 2]),
                        op=mybir.AluOpType.mult)
                    rs = rsp.tile([128, CHUNK // L, 2], f16, tag="rs")
                    nc.vector.tensor_reduce(
                        out=rs[:],
                        in_=g[:].rearrange("p (r k) two -> p r two k", k=L),
                        axis=mybir.AxisListType.X, op=mybir.AluOpType.add)
                    nc.sync.dma_start(
                        rows_d[:, s0 // L:(s0 + CHUNK) // L, :], rs[:])

    lower_extended_insts(nc)
    split_excess_waits(nc)
    return nc


def kernel(x, edge_weight, edge_index, num_nodes):
    x = np.ascontiguousarray(np.asarray(x, dtype=np.float32))
    xt4, cores, nchunks = pack_host(x, edge_weight, edge_index)
    nslots = cores[0]["nslots"]
    nc = build_bass(nchunks, nslots)
    in_maps = [
        {"xt4": xt4, "idx16": c["idx16"], "w1": c["w1"]} for c in cores
    ]
    from concourse.bass_utils import run_bass_kernel_spmd
    res = run_bass_kernel_spmd(nc, in_maps, core_ids=list(range(NCORES)))
    outs = [combine_host(res.results[c]["rows"], cores[c])
            for c in range(NCORES)]
    return np.concatenate(outs, axis=0).astype(np.float32)


# revision 9
# speedup vs baseline: 11.4862x; 1.5228x over previous
"""GNN message passing (gather + weighted scatter-add) on 8 Trainium2 cores, v2.

out[n, f] = sum over edges e with dst[e]==n of edge_weight[e] * x[src[e], f]

Architecture (driven by measured per-instruction dispatch costs of ~30-120us
on this runtime — total instruction count is everything):
  - Destination-shard: core c owns output nodes [c*12500, (c+1)*12500).
  - x lives fully SBUF-resident, fp16, transposed + quarter-partitioned:
    xT4[p, n, l] = x[(p//32)*25000 + n, 2*(p%32) + l]  -> [128, 25000, 2],
    100KB/partition. int16 ap_gather indices stay < 25000.
  - Edges packed into 8-slot rows per dst node (ceil(deg/8) rows/node).
    Slot j: ap_gather pulls x columns for src_j into every partition
    (each 16-partition group uses its own index: the src quarter that
    group holds, else a dummy); host-baked weights wt4[p, j] =
    w_j * (quarter(p) == quarter(src_j)) kill the wrong-quarter copies.
  - Per chunk (~6-7k slots): 1 ap_gather (gpsimd) + 1 broadcast multiply
    (DVE) + 1 strided row-reduce (DVE) + wt/rowsum DMAs. ~30 chunks/core,
    ~130 instructions total.
  - Row sums [128=(pair,quarter), rows, 2] stream to DRAM; the host sums
    the <=4 quarter partials and ceil(deg/8) row partials per node (O(N)
    work) and re-interleaves features.
"""

import math
import numpy as np

N = 100000
E = 1000000
F = 64
NCORES = 8
NPC = N // NCORES            # 12500 dst nodes per core
Q = 4                        # x quarters (int16 index limit)
NQ = N // Q                  # 25000
NQ1 = NQ + 1                 # +1 zero-sentinel column (gathered by dummies)
L = 4                        # slots per row
CHUNK = 7680                 # slots per chunk (multiple of 128)

MULT_MODE = "dve"            # "dve" | "gpsimd" | "split" (alternate)
REPEAT = 1                   # device-body repetitions (timing amplification)


def pack_host(x, edge_weight, edge_index):
    src = np.asarray(edge_index[0], dtype=np.int64)
    dst = np.asarray(edge_index[1], dtype=np.int64)
    w = np.asarray(edge_weight, dtype=np.float32)

    xpair = np.ascontiguousarray(
        x.astype(np.float16).reshape(N, 32, 2).transpose(1, 0, 2)
    )  # [32, N, 2]
    xt4 = np.zeros((128, NQ1, 2), dtype=np.float16)
    xt4[:, 1:, :] = (
        xpair.reshape(32, Q, NQ, 2).transpose(1, 0, 2, 3).reshape(128, NQ, 2)
    )  # partition p = (q = p//32, pair = p%32); column 0 stays zero (sentinel)

    core = dst // NPC
    cores = []
    for c in range(NCORES):
        sel = core == c
        es = src[sel]
        ed = dst[sel] - c * NPC
        ew = w[sel]
        order = np.argsort(ed, kind="stable")
        es, ed, ew = es[order], ed[order], ew[order]

        deg = np.bincount(ed, minlength=NPC)
        nrows_per_node = np.maximum((deg + L - 1) // L, 1)
        nrows = int(nrows_per_node.sum())
        # chunk layout in rows (CHUNK/L rows per chunk), pad rows to fill
        rows_per_chunk = CHUNK // L
        nchunks = math.ceil(nrows / rows_per_chunk)
        nrows_pad = nchunks * rows_per_chunk
        nslots = nrows_pad * L

        row_node = np.zeros(nrows_pad, dtype=np.int64)  # node of each row
        # rows in node order
        row_node[:nrows] = np.repeat(np.arange(NPC), nrows_per_node)
        row_node[nrows:] = 0  # pad rows -> node 0 with zero weight

        slot_src = np.zeros(nslots, dtype=np.int64)
        slot_w = np.zeros(nslots, dtype=np.float32)
        # edge positions: node n's edges go into its rows' slots in order
        row_start = np.zeros(NPC + 1, dtype=np.int64)
        row_start[1:] = np.cumsum(nrows_per_node)
        node_edge_start = np.zeros(NPC + 1, dtype=np.int64)
        node_edge_start[1:] = np.cumsum(deg)
        # position of edge within its node  (edges are dst-sorted)
        epos = np.arange(len(ed)) - node_edge_start[ed]
        slot_idx = row_start[ed] * L + epos
        slot_src[slot_idx] = es
        slot_w[slot_idx] = ew

        sq = slot_src // NQ          # quarter of each slot's src
        sl = (slot_src % NQ + 1).astype(np.int16)  # 1-based; 0 = zero sentinel

        # ap_gather index table [128, nslots/16] int16, per-16-partition group:
        # the group holding quarter q gathers its slots' rows; others gather 0s
        idx16 = np.zeros((128, nslots // 16), dtype=np.int16)
        slocal = sl.reshape(nslots // 16, 16).T  # [16, s]
        squar = sq.reshape(nslots // 16, 16).T
        for g in range(8):
            gq = g // 2
            idx16[g * 16:(g + 1) * 16, :] = np.where(squar == gq, slocal, 0)

        # per-slot weights [1, nslots] fp16 (quarter masking is redundant:
        # wrong-quarter partitions gather the zero sentinel column)
        w1 = slot_w.astype(np.float16)[None, :]

        cores.append(dict(
            idx16=idx16, w1=w1, nrows=nrows, nrows_pad=nrows_pad,
            nslots=nslots, nchunks=nchunks, row_node=row_node,
        ))
    maxchunks = max(c["nchunks"] for c in cores)
    # pad all cores to identical chunk count (single SPMD program)
    for c in cores:
        if c["nchunks"] < maxchunks:
            extra = (maxchunks - c["nchunks"]) * CHUNK
            c["idx16"] = np.concatenate(
                [c["idx16"], np.zeros((128, extra // 16), np.int16)], axis=1)
            c["w1"] = np.concatenate(
                [c["w1"], np.zeros((1, extra), np.float16)], axis=1)
            pad_rows = extra // L
            c["row_node"] = np.concatenate(
                [c["row_node"], np.zeros(pad_rows, np.int64)])
            c["nchunks"] = maxchunks
            c["nrows_pad"] += pad_rows
            c["nslots"] += extra
    return xt4, cores, maxchunks


def emulate_core(xt4, core):
    """Numpy emulation of the device program for one core."""
    nch = core["nchunks"]
    idx16, w1 = core["idx16"], core["w1"]
    rows_out = np.zeros((128, core["nrows_pad"], 2), dtype=np.float16)
    for ch in range(nch):
        s0 = ch * CHUNK
        g = np.zeros((128, CHUNK, 2), dtype=np.float16)
        for grp in range(8):
            idxs = idx16[grp * 16:(grp + 1) * 16, s0 // 16:(s0 + CHUNK) // 16]
            flat = idxs.T.reshape(-1).astype(np.int64)  # slot order
            g[grp * 16:(grp + 1) * 16] = xt4[grp * 16:(grp + 1) * 16, flat, :]
        wc = w1[0, s0:s0 + CHUNK, None].astype(np.float32)
        gw = (g.astype(np.float32) * wc[None]).astype(np.float16)
        r = gw.astype(np.float32).reshape(128, CHUNK // L, L, 2).sum(axis=2)
        rows_out[:, s0 // L:(s0 + CHUNK) // L, :] = r.astype(np.float16)
    return rows_out


def combine_host(rows_out, core):
    """rows [128=(q,pair), rows, 2] fp16 -> [12500, 64] f32 for one core."""
    r = rows_out.astype(np.float32)  # [128, R, 2]
    rq = r.reshape(Q, 32, -1, 2).sum(axis=0)  # [32, R, 2]
    out = np.zeros((NPC, 32, 2), dtype=np.float32)
    np.add.at(out, core["row_node"], rq.transpose(1, 0, 2))
    return out.reshape(NPC, F)


WAIT_CAPS = {"InstEventSemaphore": 8}


def split_excess_waits(nc):
    """Walrus only encodes one sync wait per instruction; move the excess
    onto standalone InstEventSemaphore instructions placed just before."""
    import concourse.mybir as mybir
    n = 0
    for f in nc.m.functions:
        for bb in f.blocks:
            eng_ids = {}
            new = []
            for ins in bb.instructions:
                si = ins.sync_info
                waits = list(si.on_wait) if (si is not None and si.on_wait) else []
                cap = WAIT_CAPS.get(type(ins).__name__, 1)
                if len(waits) > cap:
                    excess, keep = waits[:-cap], waits[-cap:]
                    if ins.engine not in eng_ids:
                        eng_ids[ins.engine] = 245 + len(eng_ids)
                    sem_id = eng_ids[ins.engine]
                    sem_name = f"esw_scratch_{sem_id}"
                    for wchunk in [excess[i:i + 1] for i in range(len(excess))]:
                        n += 1
                        upd = mybir.SyncUpdate(
                            sync_type="semaphore", id=sem_id, ant_name=sem_name,
                            update_mode="sem-add-imm", update_value=0,
                        )
                        es = mybir.InstEventSemaphore(
                            name=f"ESW-{n}-{ins.name}",
                            engine=ins.engine,
                            ins=[], outs=[],
                            sync_info=mybir.SyncInfo(on_wait=wchunk, on_update=[upd]),
                        )
                        new.append(es)
                    si.on_wait = keep
                new.append(ins)
            bb.instructions = new
    return n


_walrus_patched = False


def patch_walrus_dge():
    global _walrus_patched
    if _walrus_patched:
        return
    import concourse.bass_utils as bu
    orig = bu.run_command

    def run_command_dge(argv, **kw):
        argv = list(argv)
        if argv and "walrus_driver" in str(argv[0]) and not any(
                str(a).startswith("--dge-levels") for a in argv):
            argv.append("--dge-levels=vector_dynamic_offsets")
        return orig(argv, **kw)

    bu.run_command = run_command_dge
    _walrus_patched = True


def build_bass(nchunks, nslots):
    import concourse.bass as bass
    import concourse.mybir as mybir
    import concourse.tile as tile
    from concourse import library_config
    from concourse.library_overlay import lower_extended_insts

    patch_walrus_dge()
    f16, i16 = mybir.dt.float16, mybir.dt.int16
    nrows_pad = nslots // L

    nc = bass.Bass("TRN2")
    xt4_d = nc.dram_tensor("xt4", [128, NQ1, 2], f16, kind="ExternalInput")
    idx_d = nc.dram_tensor("idx16", [128, nslots // 16], i16, kind="ExternalInput")
    w1_d = nc.dram_tensor("w1", [1, nslots], f16, kind="ExternalInput")
    rows_d = nc.dram_tensor("rows", [128, nrows_pad, 2], f16, kind="ExternalOutput")

    with nc.allow_low_precision("fp16 8-term row sums; host combines in f32"):
      with tile.TileContext(nc, pool_alloc_mode="queue") as tc:
        with (
            tc.tile_pool(name="const", bufs=1) as constp,
            tc.tile_pool(name="g", bufs=2) as gp,
            tc.tile_pool(name="wt", bufs=1) as wtp,
            tc.tile_pool(name="rs", bufs=2) as rsp,
        ):
            xt4_sb = constp.tile([128, NQ1, 2], f16, tag="xt4")
            nc.sync.dma_start(xt4_sb[:], xt4_d[:])
            idx_sb = constp.tile([128, nslots // 16], i16, tag="idx")
            nc.sync.dma_start(idx_sb[:], idx_d[:])
            nc.gpsimd.load_library(library_config.ap_gather)

            for _rep in range(REPEAT):
                for ch in range(nchunks):
                    s0 = ch * CHUNK
                    g = gp.tile([128, CHUNK, 2], f16, tag="g")
                    nc.gpsimd.ap_gather(
                        g[:], xt4_sb[:], idx_sb[:, s0 // 16:(s0 + CHUNK) // 16],
                        channels=128, num_elems=NQ1, d=2, num_idxs=CHUNK)
                    wtc = wtp.tile([128, CHUNK], f16, tag="wt")
                    nc.scalar.dma_start(
                        wtc[:], w1_d[0:1, s0:s0 + CHUNK].broadcast_to([128, CHUNK]))
                    if MULT_MODE == "dve":
                        mult_eng = nc.vector
                    elif MULT_MODE == "gpsimd":
                        mult_eng = nc.gpsimd
                    else:
                        mult_eng = nc.vector if ch % 2 == 0 else nc.gpsimd
                    mult_eng.tensor_tensor(
                        out=g[:], in0=g[:],
                        in1=wtc[:].unsqueeze(2).broadcast_to([128, CHUNK, 2]),
                        op=mybir.AluOpType.mult)
                    rs = rsp.tile([128, CHUNK // L, 2], f16, tag="rs")
                    nc.vector.tensor_reduce(
                        out=rs[:],
                        in_=g[:].rearrange("p (r k) two -> p r two k", k=L),
                        axis=mybir.AxisListType.X, op=mybir.AluOpType.add)
                    nc.sync.dma_start(
                        rows_d[:, s0 // L:(s0 + CHUNK) // L, :], rs[:])

    lower_extended_insts(nc)
    split_excess_waits(nc)
    return nc


def kernel(x, edge_weight, edge_index, num_nodes):
    x = np.ascontiguousarray(np.asarray(x, dtype=np.float32))
    xt4, cores, nchunks = pack_host(x, edge_weight, edge_index)
    nslots = cores[0]["nslots"]
    nc = build_bass(nchunks, nslots)
    in_maps = [
        {"xt4": xt4, "idx16": c["idx16"], "w1": c["w1"]} for c in cores
    ]
    from concourse.bass_utils import run_bass_kernel_spmd
    res = run_bass_kernel_spmd(nc, in_maps, core_ids=list(range(NCORES)))
    outs = [combine_host(res.results[c]["rows"], cores[c])
            for c in range(NCORES)]
    return np.concatenate(outs, axis=0).astype(np.float32)
